# revision 19
# baseline (speedup 1.0000x reference)
"""Trainium2 Bass kernel for nn_Attention_9612136809120.

Reference math (B=1, H=8, S=4096, D=64, fp32):
    s[b,h,q,k] = q . k
    s = where(mask[q] & mask[k], s, -1e20)
    A = softmax(s, axis=q)            # NOTE: normalized over the QUERY axis
    out[b,h,q,d] = sum_k A[q,k] v[k,d]

Device strategy (8 cores):
  8 rounds, one head per round; all 8 cores cooperate on the head with an
  8-way split over k. Core c owns k-slice [c*512, (c+1)*512).

  Column-softmax (over q) with contraction over k means the normalizer
  l[k] = sum_q exp(s[q,k]) is local to a k-slice, so each core can fully
  normalize its partial output; partial outputs (over k-slices) are summed
  with a ReduceScatter across the 8 cores.

  Masking is folded into the matmul by augmenting the contraction dim with
  two extra rows so s~ = s + c_q + c_k with c = 0 (keep) / -1e20 (masked).
  exp(s~) is then exactly 0 in masked rows/cols. Columns with mask[k]=False
  (which the reference turns into uniform 1/S over *all* q) are handled
  exactly by an analytic bias term b[d] = (1/S) * sum_{masked k} v[k,d],
  added to every output row, while r[k] = mask[k]/l[k] zeroes their normal
  contribution. No max-subtraction is needed: |s| < ~60 so exp never
  overflows, and softmax is shift-invariant so values match the reference
  to fp32 rounding.

  Score matmul runs as a bf16 hi/lo split (2 matmuls, error ~2^-16):
     s ~= kh.(qh+ql) + kl.qh   (dropping kl.ql)
  MM_A: lhsT rows [kh;kh] (K=128), rhs rows [qh;ql]
  MM_B: lhsT rows [kl;ones;c_k] (K=66), rhs rows [qh;c_q;ones]
  The output matmul uses float32r (fp32 rounded to 11 mantissa bits,
  1 cyc/col on the PE); its rounding error averages out over the 4096-term
  contraction.

  Per round, per core:
    pass 1: P[k,q] = exp(s~)  (PE -> PSUM, ACT exp with accum_out -> l)
    r = mask/l; v'' = v * r; bias b via tiny matmul
    pass 2: outT_partial[d,q] = sum_k v''[k,d] * P[k,q] + b
    ReduceScatter(add) over 8 cores -> each core gets 8 rows of outT[64,4096]

Host side only reshapes/transposes/casts for sharding and gathers output.
"""

import os

import numpy as np
import ml_dtypes

import concourse.bass as bass
import concourse.tile as tile
from concourse import bacc, mybir
from concourse.bass_utils import run_bass_kernel_spmd

B, H, S, D = 1, 8, 4096, 64
NEG = -1e20
N_CORES = 8
KSLICE = S // N_CORES          # 512 k rows per core per round
KC = KSLICE // 128             # 4 k-chunks of 128
DA = D + 2                     # kl/qh + mask aug rows
ROUNDS = H                     # one head per round
QTILE = 1024                   # ACT exp tile width (2 PSUM banks)
NQ = S // QTILE                # exp tiles per k-chunk row-block
FP32 = mybir.dt.float32
F32R = mybir.dt.float32r
BF16 = mybir.dt.bfloat16


def build_bass():
    nc = bacc.Bacc("TRN2", target_bir_lowering=False, debug=False,
                   num_devices=N_CORES)

    qa = nc.dram_tensor("qa", [ROUNDS, 128, S], BF16, kind="ExternalInput")
    qb = nc.dram_tensor("qb", [ROUNDS, DA, S], BF16, kind="ExternalInput")
    ka = nc.dram_tensor("ka", [ROUNDS, 128, KSLICE], BF16, kind="ExternalInput")
    kb = nc.dram_tensor("kb", [ROUNDS, DA, KSLICE], BF16, kind="ExternalInput")
    vm = nc.dram_tensor("vm", [ROUNDS, 128, KC, D + 3], FP32,
                        kind="ExternalInput")
    outp = nc.dram_tensor(
        "outp", [ROUNDS, D * S // N_CORES], FP32, kind="ExternalOutput"
    )

    ccin = nc.dram_tensor("ccin", [ROUNDS, D, S], FP32)
    ccout = nc.dram_tensor("ccout", [ROUNDS, D * S // N_CORES], FP32)

    with tile.TileContext(nc) as tc:
        with (
            tc.tile_pool(name="qp", bufs=2) as qp,
            tc.tile_pool(name="kp", bufs=2) as kp,
            tc.tile_pool(name="vp", bufs=2) as vp,
            tc.tile_pool(name="v2p", bufs=2) as v2p,
            tc.tile_pool(name="mp", bufs=2) as mp,
            tc.tile_pool(name="pp", bufs=2) as pp,
            tc.tile_pool(name="statp", bufs=2) as statp,
            tc.tile_pool(name="outp_sb", bufs=1) as outp_sb,
            tc.tile_pool(name="ps_s", bufs=3, space="PSUM") as ps_s,
            tc.tile_pool(name="ps_o", bufs=1, space="PSUM") as ps_o,
            tc.tile_pool(name="ps_b", bufs=1, space="PSUM") as ps_b,
        ):
            def emit_pass1_units(r):
                qa_t = qp.tile([128, S], BF16, tag="qa")
                qb_t = qp.tile([DA, S], BF16, tag="qb")
                for ch in range(4):
                    cs = slice(ch * (S // 4), (ch + 1) * (S // 4))
                    nc.sync.dma_start(out=qa_t[:, cs], in_=qa[r, :, cs])
                    nc.sync.dma_start(out=qb_t[:, cs], in_=qb[r, :, cs])
                ka_t = kp.tile([128, KSLICE], BF16, tag="ka")
                nc.sync.dma_start(out=ka_t, in_=ka[r])
                kb_t = kp.tile([DA, KSLICE], BF16, tag="kb")
                nc.sync.dma_start(out=kb_t, in_=kb[r])
                vm_t = vp.tile([128, KC, D + 3], FP32, tag="vm")
                nc.sync.dma_start(out=vm_t, in_=vm[r])
                v_t = vm_t[:, :, :D]
                mf_t = vm_t[:, :, D]
                ma_t = vm_t[:, :, D + 1]
                mc_t = vm_t[:, :, D + 2]

                p_t = pp.tile([128, KC, S], F32R, tag="P")
                lpart = statp.tile([128, KC, NQ], FP32, tag="lpart")
                state = {}
                for kc in range(KC):
                    ksl = slice(kc * 128, (kc + 1) * 128)
                    for j in range(NQ):
                        ps = ps_s.tile([128, QTILE], FP32, tag="s")
                        for h2 in range(QTILE // 512):
                            q0 = j * QTILE + h2 * 512
                            out_sl = ps[:, h2 * 512:(h2 + 1) * 512]
                            nc.tensor.matmul(
                                out_sl,
                                lhsT=ka_t[:, ksl],
                                rhs=qa_t[:, q0:q0 + 512],
                                start=True,
                                stop=False,
                            )
                            nc.tensor.matmul(
                                out_sl,
                                lhsT=kb_t[:, ksl],
                                rhs=qb_t[:, q0:q0 + 512],
                                start=False,
                                stop=True,
                            )
                        nc.scalar.activation(
                            p_t[:, kc, j * QTILE:(j + 1) * QTILE],
                            ps,
                            mybir.ActivationFunctionType.Exp,
                            accum_out=lpart[:, kc, j:j + 1],
                        )
                        yield None

                # normalizer: r = mask / l  (masked k -> 0)
                l_t = statp.tile([128, KC], FP32, tag="l")
                nc.vector.tensor_reduce(
                    l_t, lpart, axis=mybir.AxisListType.X, op=mybir.AluOpType.add
                )
                denom = statp.tile([128, KC], FP32, tag="denom")
                nc.vector.tensor_add(denom, l_t, ma_t)  # masked k: l=0 -> denom=1
                rec = statp.tile([128, KC], FP32, tag="rec")
                nc.vector.reciprocal(rec, denom)
                r_t = statp.tile([128, KC], FP32, tag="r")
                nc.vector.tensor_mul(r_t, rec, mf_t)

                v2_t = v2p.tile([128, KC, D], F32R, tag="v2")
                for kc in range(KC):
                    nc.vector.tensor_scalar_mul(
                        v2_t[:, kc, :], v_t[:, kc, :], r_t[:, kc:kc + 1]
                    )

                # bias: b[d] = sum_{local masked k} v[k,d] / S
                pb = ps_b.tile([D, 1], FP32, tag="b")
                for kc in range(KC):
                    nc.tensor.matmul(
                        pb,
                        lhsT=v_t[:, kc, :],
                        rhs=mc_t[:, kc:kc + 1],
                        start=(kc == 0),
                        stop=(kc == KC - 1),
                    )
                b_sb = statp.tile([D, 1], FP32, tag="bsb")
                nc.vector.tensor_copy(b_sb, pb)
                state["pvb"] = (p_t, v2_t, b_sb)
                yield state

            def emit_pass2_units(r, state):
                p_t, v2_t, b_sb = state["pvb"]
                out_t = outp_sb.tile([D, S], FP32, tag="out")
                for qq in range(S // 512):
                    po = ps_o.tile([D, 512], FP32, tag="o")
                    for kc in range(KC):
                        nc.tensor.matmul(
                            po,
                            lhsT=v2_t[:, kc, :],
                            rhs=p_t[:, kc, qq * 512:(qq + 1) * 512],
                            start=(kc == 0),
                            stop=(kc == KC - 1),
                        )
                    nc.vector.tensor_scalar_add(
                        out_t[:, qq * 512:(qq + 1) * 512], po, b_sb
                    )
                    yield None

                nc.sync.dma_start(out=ccin[r], in_=out_t)
                nc.gpsimd.collective_compute(
                    "ReduceScatter",
                    mybir.AluOpType.add,
                    replica_groups=[list(range(N_CORES))],
                    ins=[ccin[r]],
                    outs=[ccout[r]],
                )
                nc.sync.dma_start(out=outp[r], in_=ccout[r])
                yield None

            # software pipeline with fine-grained interleave: round r's score
            # matmuls (bf16, HAM-visible) are woven between round r-1's output
            # matmuls (fp32r, which do not register as PE activity) so the
            # clock gate stays warm.
            def drain(gen):
                for _ in gen:
                    pass

            prev_state = None
            for r in range(ROUNDS):
                g1 = emit_pass1_units(r)
                g2 = (
                    emit_pass2_units(r - 1, prev_state)
                    if prev_state is not None else None
                )
                state = None
                n1 = 0
                while True:
                    item = next(g1, "DONE")
                    if item == "DONE":
                        break
                    if isinstance(item, dict):
                        state = item
                    n1 += 1
                    if g2 is not None and n1 % 2 == 0:
                        next(g2, None)
                if g2 is not None:
                    drain(g2)
                prev_state = state
            drain(emit_pass2_units(ROUNDS - 1, prev_state))

    nc.finalize()
    return nc


_NC_CACHE = None


def _get_nc():
    global _NC_CACHE
    if _NC_CACHE is None:
        _NC_CACHE = build_bass()
    return _NC_CACHE


def kernel(q, k, v, attn_mask):
    q = np.asarray(q, dtype=np.float32)
    k = np.asarray(k, dtype=np.float32)
    v = np.asarray(v, dtype=np.float32)
    mask = np.asarray(attn_mask).astype(bool)[0]          # [S]

    bf = ml_dtypes.bfloat16
    maskf = mask.astype(np.float32)                        # 1.0 keep / 0.0 drop
    cvec = np.where(mask, np.float32(0.0), np.float32(NEG)).astype(bf)
    ones = np.ones(S, bf)

    qt = np.ascontiguousarray(q[0].transpose(0, 2, 1))     # [H, D, S] fp32
    kt = np.ascontiguousarray(k[0].transpose(0, 2, 1))
    qh = qt.astype(bf)
    ql = (qt - qh.astype(np.float32)).astype(bf)
    kh = kt.astype(bf)
    kl = (kt - kh.astype(np.float32)).astype(bf)

    qa = np.concatenate([qh, ql], axis=1)                  # [H, 128, S]
    qb = np.empty((H, DA, S), bf)
    qb[:, :D] = qh
    qb[:, D] = cvec
    qb[:, D + 1] = ones
    ka_full = np.concatenate([kh, kh], axis=1)             # [H, 128, S]
    kb_full = np.empty((H, DA, S), bf)
    kb_full[:, :D] = kl
    kb_full[:, D] = ones
    kb_full[:, D + 1] = cvec

    mf_full = maskf.reshape(N_CORES, KC, 128).transpose(0, 2, 1)   # [8,128,KC]
    ma_full = 1.0 - mf_full
    mc_full = ma_full / np.float32(S)

    in_maps = []
    for c in range(N_CORES):
        sl = slice(c * KSLICE, (c + 1) * KSLICE)
        # packed [H, 128, KC, D+3]: v | maskf | 1-maskf | (1-maskf)/S
        vmc = np.empty((H, 128, KC, D + 3), np.float32)
        vmc[..., :D] = v[0, :, sl, :].reshape(H, KC, 128, D).transpose(0, 2, 1, 3)
        vmc[..., D] = mf_full[c][None]
        vmc[..., D + 1] = ma_full[c][None]
        vmc[..., D + 2] = mc_full[c][None]
        in_maps.append({
            "qa": qa,
            "qb": qb,
            "ka": np.ascontiguousarray(ka_full[:, :, sl]),
            "kb": np.ascontiguousarray(kb_full[:, :, sl]),
            "vm": vmc,
        })

    nc = _get_nc()
    trace = bool(os.environ.get("KERNEL_TRACE"))
    res = run_bass_kernel_spmd(
        nc, in_maps, list(range(N_CORES)),
        trace=trace,
        tmpdir=os.environ.get("KERNEL_TRACE_DIR") or None,
    )
    if trace and res.exec_time_ns is not None:
        print(f"HW exec time: {res.exec_time_ns} ns")

    # gather: core c holds rows [8c, 8c+8) of outT[h] = [D, S]
    out = np.empty((B, H, S, D), np.float32)
    rows = D // N_CORES
    for h in range(H):
        out_t = np.empty((D, S), np.float32)
        for c in range(N_CORES):
            out_t[c * rows:(c + 1) * rows] = (
                res.results[c]["outp"][h].reshape(rows, S)
            )
        out[0, h] = out_t.T
    return out


# revision 20
# speedup vs baseline: 1.0424x; 1.0424x over previous
"""Trainium2 Bass kernel for nn_Attention_9612136809120.

Reference math (B=1, H=8, S=4096, D=64, fp32):
    s[b,h,q,k] = q . k
    s = where(mask[q] & mask[k], s, -1e20)
    A = softmax(s, axis=q)            # NOTE: normalized over the QUERY axis
    out[b,h,q,d] = sum_k A[q,k] v[k,d]

Device strategy (8 cores):
  8 rounds, one head per round; all 8 cores cooperate on the head with an
  8-way split over k. Core c owns k-slice [c*512, (c+1)*512).

  Column-softmax (over q) with contraction over k means the normalizer
  l[k] = sum_q exp(s[q,k]) is local to a k-slice, so each core can fully
  normalize its partial output; partial outputs (over k-slices) are summed
  with a ReduceScatter across the 8 cores.

  Masking is folded into the matmul by augmenting the contraction dim with
  two extra rows so s~ = s + c_q + c_k with c = 0 (keep) / -1e20 (masked).
  exp(s~) is then exactly 0 in masked rows/cols. Columns with mask[k]=False
  (which the reference turns into uniform 1/S over *all* q) are handled
  exactly by an analytic bias term b[d] = (1/S) * sum_{masked k} v[k,d],
  added to every output row, while r[k] = mask[k]/l[k] zeroes their normal
  contribution. No max-subtraction is needed: |s| < ~60 so exp never
  overflows, and softmax is shift-invariant so values match the reference
  to fp32 rounding.

  Score matmul runs as a bf16 hi/lo split (2 matmuls, error ~2^-16):
     s ~= kh.(qh+ql) + kl.qh   (dropping kl.ql)
  MM_A: lhsT rows [kh;kh] (K=128), rhs rows [qh;ql]
  MM_B: lhsT rows [kl;ones;c_k] (K=66), rhs rows [qh;c_q;ones]
  The output matmul uses float32r (fp32 rounded to 11 mantissa bits,
  1 cyc/col on the PE); its rounding error averages out over the 4096-term
  contraction.

  Per round, per core:
    pass 1: P[k,q] = exp(s~)  (PE -> PSUM, ACT exp with accum_out -> l)
    r = mask/l; v'' = v * r; bias b via tiny matmul
    pass 2: outT_partial[d,q] = sum_k v''[k,d] * P[k,q] + b
    ReduceScatter(add) over 8 cores -> each core gets 8 rows of outT[64,4096]

Host side only reshapes/transposes/casts for sharding and gathers output.
"""

import os

import numpy as np
import ml_dtypes

import concourse.bass as bass
import concourse.tile as tile
from concourse import bacc, mybir
from concourse.bass_utils import run_bass_kernel_spmd

B, H, S, D = 1, 8, 4096, 64
NEG = -1e20
N_CORES = 8
KSLICE = S // N_CORES          # 512 k rows per core per round
KC = KSLICE // 128             # 4 k-chunks of 128
DA = D + 2                     # kl/qh + mask aug rows
ROUNDS = H                     # one head per round
QTILE = 1024                   # ACT exp tile width (2 PSUM banks)
NQ = S // QTILE                # exp tiles per k-chunk row-block
FP32 = mybir.dt.float32
F32R = mybir.dt.float32r
BF16 = mybir.dt.bfloat16


def build_bass():
    nc = bacc.Bacc("TRN2", target_bir_lowering=False, debug=False,
                   num_devices=N_CORES)

    qa = nc.dram_tensor("qa", [ROUNDS, 128, S], BF16, kind="ExternalInput")
    qb = nc.dram_tensor("qb", [ROUNDS, DA, S], BF16, kind="ExternalInput")
    ka = nc.dram_tensor("ka", [ROUNDS, 128, KSLICE], BF16, kind="ExternalInput")
    kb = nc.dram_tensor("kb", [ROUNDS, DA, KSLICE], BF16, kind="ExternalInput")
    vm = nc.dram_tensor("vm", [ROUNDS, 128, KC, D + 3], FP32,
                        kind="ExternalInput")
    outp = nc.dram_tensor(
        "outp", [ROUNDS, D * S // N_CORES], FP32, kind="ExternalOutput"
    )

    ccin = nc.dram_tensor("ccin", [ROUNDS, D, S], FP32)
    ccout = nc.dram_tensor("ccout", [ROUNDS, D * S // N_CORES], FP32)

    with tile.TileContext(nc) as tc:
        with (
            tc.tile_pool(name="qp", bufs=2) as qp,
            tc.tile_pool(name="kp", bufs=2) as kp,
            tc.tile_pool(name="vp", bufs=2) as vp,
            tc.tile_pool(name="v2p", bufs=2) as v2p,
            tc.tile_pool(name="mp", bufs=2) as mp,
            tc.tile_pool(name="pp", bufs=2) as pp,
            tc.tile_pool(name="statp", bufs=2) as statp,
            tc.tile_pool(name="outp_sb", bufs=1) as outp_sb,
            tc.tile_pool(name="ps_s", bufs=2, space="PSUM") as ps_s,
            tc.tile_pool(name="ps_o", bufs=2, space="PSUM") as ps_o,
            tc.tile_pool(name="ps_b", bufs=1, space="PSUM") as ps_b,
        ):
            def emit_pass1_units(r):
                qa_t = qp.tile([128, S], BF16, tag="qa")
                nc.sync.dma_start(out=qa_t, in_=qa[r])
                qb_t = qp.tile([DA, S], BF16, tag="qb")
                nc.sync.dma_start(out=qb_t, in_=qb[r])
                ka_t = kp.tile([128, KSLICE], BF16, tag="ka")
                nc.sync.dma_start(out=ka_t, in_=ka[r])
                kb_t = kp.tile([DA, KSLICE], BF16, tag="kb")
                nc.sync.dma_start(out=kb_t, in_=kb[r])
                vm_t = vp.tile([128, KC, D + 3], FP32, tag="vm")
                nc.sync.dma_start(out=vm_t, in_=vm[r])
                v_t = vm_t[:, :, :D]
                mf_t = vm_t[:, :, D]
                ma_t = vm_t[:, :, D + 1]
                mc_t = vm_t[:, :, D + 2]

                p_t = pp.tile([128, KC, S], F32R, tag="P")
                lpart = statp.tile([128, KC, NQ], FP32, tag="lpart")
                state = {}
                for kc in range(KC):
                    ksl = slice(kc * 128, (kc + 1) * 128)
                    for j in range(NQ):
                        ps = ps_s.tile([128, QTILE], FP32, tag="s")
                        for h2 in range(QTILE // 512):
                            q0 = j * QTILE + h2 * 512
                            out_sl = ps[:, h2 * 512:(h2 + 1) * 512]
                            nc.tensor.matmul(
                                out_sl,
                                lhsT=ka_t[:, ksl],
                                rhs=qa_t[:, q0:q0 + 512],
                                start=True,
                                stop=False,
                            )
                            nc.tensor.matmul(
                                out_sl,
                                lhsT=kb_t[:, ksl],
                                rhs=qb_t[:, q0:q0 + 512],
                                start=False,
                                stop=True,
                            )
                        nc.scalar.activation(
                            p_t[:, kc, j * QTILE:(j + 1) * QTILE],
                            ps,
                            mybir.ActivationFunctionType.Exp,
                            accum_out=lpart[:, kc, j:j + 1],
                        )
                        yield None

                # normalizer: r = mask / l  (masked k -> 0)
                l_t = statp.tile([128, KC], FP32, tag="l")
                nc.vector.tensor_reduce(
                    l_t, lpart, axis=mybir.AxisListType.X, op=mybir.AluOpType.add
                )
                denom = statp.tile([128, KC], FP32, tag="denom")
                nc.vector.tensor_add(denom, l_t, ma_t)  # masked k: l=0 -> denom=1
                rec = statp.tile([128, KC], FP32, tag="rec")
                nc.vector.reciprocal(rec, denom)
                r_t = statp.tile([128, KC], FP32, tag="r")
                nc.vector.tensor_mul(r_t, rec, mf_t)

                v2_t = v2p.tile([128, KC, D], F32R, tag="v2")
                for kc in range(KC):
                    nc.vector.tensor_scalar_mul(
                        v2_t[:, kc, :], v_t[:, kc, :], r_t[:, kc:kc + 1]
                    )

                # bias: b[d] = sum_{local masked k} v[k,d] / S
                pb = ps_b.tile([D, 1], FP32, tag="b")
                for kc in range(KC):
                    nc.tensor.matmul(
                        pb,
                        lhsT=v_t[:, kc, :],
                        rhs=mc_t[:, kc:kc + 1],
                        start=(kc == 0),
                        stop=(kc == KC - 1),
                    )
                b_sb = statp.tile([D, 1], FP32, tag="bsb")
                nc.vector.tensor_copy(b_sb, pb)
                state["pvb"] = (p_t, v2_t, b_sb)
                yield state

            def emit_pass2_units(r, state):
                p_t, v2_t, b_sb = state["pvb"]
                out_t = outp_sb.tile([D, S], FP32, tag="out")
                for qq in range(S // 512):
                    po = ps_o.tile([D, 512], FP32, tag="o")
                    for kc in range(KC):
                        nc.tensor.matmul(
                            po,
                            lhsT=v2_t[:, kc, :],
                            rhs=p_t[:, kc, qq * 512:(qq + 1) * 512],
                            start=(kc == 0),
                            stop=(kc == KC - 1),
                        )
                    nc.vector.tensor_scalar_add(
                        out_t[:, qq * 512:(qq + 1) * 512], po, b_sb
                    )
                    yield None

                nc.sync.dma_start(out=ccin[r], in_=out_t)
                nc.gpsimd.collective_compute(
                    "ReduceScatter",
                    mybir.AluOpType.add,
                    replica_groups=[list(range(N_CORES))],
                    ins=[ccin[r]],
                    outs=[ccout[r]],
                )
                nc.sync.dma_start(out=outp[r], in_=ccout[r])
                yield None

            # software pipeline with fine-grained interleave: round r's score
            # matmuls (bf16, HAM-visible) are woven between round r-1's output
            # matmuls (fp32r, which do not register as PE activity) so the
            # clock gate stays warm.
            def drain(gen):
                for _ in gen:
                    pass

            prev_state = None
            for r in range(ROUNDS):
                g1 = emit_pass1_units(r)
                g2 = (
                    emit_pass2_units(r - 1, prev_state)
                    if prev_state is not None else None
                )
                state = None
                n1 = 0
                while True:
                    item = next(g1, "DONE")
                    if item == "DONE":
                        break
                    if isinstance(item, dict):
                        state = item
                    n1 += 1
                    if g2 is not None and n1 % 2 == 0:
                        next(g2, None)
                if g2 is not None:
                    drain(g2)
                prev_state = state
            drain(emit_pass2_units(ROUNDS - 1, prev_state))

    nc.finalize()
    return nc


_NC_CACHE = None


def _get_nc():
    global _NC_CACHE
    if _NC_CACHE is None:
        _NC_CACHE = build_bass()
    return _NC_CACHE


def kernel(q, k, v, attn_mask):
    q = np.asarray(q, dtype=np.float32)
    k = np.asarray(k, dtype=np.float32)
    v = np.asarray(v, dtype=np.float32)
    mask = np.asarray(attn_mask).astype(bool)[0]          # [S]

    bf = ml_dtypes.bfloat16
    maskf = mask.astype(np.float32)                        # 1.0 keep / 0.0 drop
    cvec = np.where(mask, np.float32(0.0), np.float32(NEG)).astype(bf)
    ones = np.ones(S, bf)

    qt = np.ascontiguousarray(q[0].transpose(0, 2, 1))     # [H, D, S] fp32
    kt = np.ascontiguousarray(k[0].transpose(0, 2, 1))
    qh = qt.astype(bf)
    ql = (qt - qh.astype(np.float32)).astype(bf)
    kh = kt.astype(bf)
    kl = (kt - kh.astype(np.float32)).astype(bf)

    qa = np.concatenate([qh, ql], axis=1)                  # [H, 128, S]
    qb = np.empty((H, DA, S), bf)
    qb[:, :D] = qh
    qb[:, D] = cvec
    qb[:, D + 1] = ones
    ka_full = np.concatenate([kh, kh], axis=1)             # [H, 128, S]
    kb_full = np.empty((H, DA, S), bf)
    kb_full[:, :D] = kl
    kb_full[:, D] = ones
    kb_full[:, D + 1] = cvec

    mf_full = maskf.reshape(N_CORES, KC, 128).transpose(0, 2, 1)   # [8,128,KC]
    ma_full = 1.0 - mf_full
    mc_full = ma_full / np.float32(S)

    in_maps = []
    for c in range(N_CORES):
        sl = slice(c * KSLICE, (c + 1) * KSLICE)
        # packed [H, 128, KC, D+3]: v | maskf | 1-maskf | (1-maskf)/S
        vmc = np.empty((H, 128, KC, D + 3), np.float32)
        vmc[..., :D] = v[0, :, sl, :].reshape(H, KC, 128, D).transpose(0, 2, 1, 3)
        vmc[..., D] = mf_full[c][None]
        vmc[..., D + 1] = ma_full[c][None]
        vmc[..., D + 2] = mc_full[c][None]
        in_maps.append({
            "qa": qa,
            "qb": qb,
            "ka": np.ascontiguousarray(ka_full[:, :, sl]),
            "kb": np.ascontiguousarray(kb_full[:, :, sl]),
            "vm": vmc,
        })

    nc = _get_nc()
    trace = bool(os.environ.get("KERNEL_TRACE"))
    res = run_bass_kernel_spmd(
        nc, in_maps, list(range(N_CORES)),
        trace=trace,
        tmpdir=os.environ.get("KERNEL_TRACE_DIR") or None,
    )
    if trace and res.exec_time_ns is not None:
        print(f"HW exec time: {res.exec_time_ns} ns")

    # gather: core c holds rows [8c, 8c+8) of outT[h] = [D, S]
    out = np.empty((B, H, S, D), np.float32)
    rows = D // N_CORES
    for h in range(H):
        out_t = np.empty((D, S), np.float32)
        for c in range(N_CORES):
            out_t[c * rows:(c + 1) * rows] = (
                res.results[c]["outp"][h].reshape(rows, S)
            )
        out[0, h] = out_t.T
    return out


# revision 21
# speedup vs baseline: 1.0554x; 1.0124x over previous
"""Trainium2 Bass kernel for nn_Attention_9612136809120.

Reference math (B=1, H=8, S=4096, D=64, fp32):
    s[b,h,q,k] = q . k
    s = where(mask[q] & mask[k], s, -1e20)
    A = softmax(s, axis=q)            # NOTE: normalized over the QUERY axis
    out[b,h,q,d] = sum_k A[q,k] v[k,d]

Device strategy (8 cores):
  8 rounds, one head per round; all 8 cores cooperate on the head with an
  8-way split over k. Core c owns k-slice [c*512, (c+1)*512).

  Column-softmax (over q) with contraction over k means the normalizer
  l[k] = sum_q exp(s[q,k]) is local to a k-slice, so each core can fully
  normalize its partial output; partial outputs (over k-slices) are summed
  with a ReduceScatter across the 8 cores.

  Masking is folded into the matmul by augmenting the contraction dim with
  two extra rows so s~ = s + c_q + c_k with c = 0 (keep) / -1e20 (masked).
  exp(s~) is then exactly 0 in masked rows/cols. Columns with mask[k]=False
  (which the reference turns into uniform 1/S over *all* q) are handled
  exactly by an analytic bias term b[d] = (1/S) * sum_{masked k} v[k,d],
  added to every output row, while r[k] = mask[k]/l[k] zeroes their normal
  contribution. No max-subtraction is needed: |s| < ~60 so exp never
  overflows, and softmax is shift-invariant so values match the reference
  to fp32 rounding.

  Score matmul runs as a bf16 hi/lo split (2 matmuls, error ~2^-16):
     s ~= kh.(qh+ql) + kl.qh   (dropping kl.ql)
  MM_A: lhsT rows [kh;kh] (K=128), rhs rows [qh;ql]
  MM_B: lhsT rows [kl;ones;c_k] (K=66), rhs rows [qh;c_q;ones]
  The output matmul uses float32r (fp32 rounded to 11 mantissa bits,
  1 cyc/col on the PE); its rounding error averages out over the 4096-term
  contraction.

  Per round, per core:
    pass 1: P[k,q] = exp(s~)  (PE -> PSUM, ACT exp with accum_out -> l)
    r = mask/l; v'' = v * r; bias b via tiny matmul
    pass 2: outT_partial[d,q] = sum_k v''[k,d] * P[k,q] + b
    ReduceScatter(add) over 8 cores -> each core gets 8 rows of outT[64,4096]

Host side only reshapes/transposes/casts for sharding and gathers output.
"""

import os

import numpy as np
import ml_dtypes

import concourse.bass as bass
import concourse.tile as tile
from concourse import bacc, mybir
from concourse.bass_utils import run_bass_kernel_spmd

B, H, S, D = 1, 8, 4096, 64
NEG = -1e20
N_CORES = 8
KSLICE = S // N_CORES          # 512 k rows per core per round
KC = KSLICE // 128             # 4 k-chunks of 128
DA = D + 2                     # kl/qh + mask aug rows
ROUNDS = H                     # one head per round
QTILE = 1024                   # ACT exp tile width (2 PSUM banks)
NQ = S // QTILE                # exp tiles per k-chunk row-block
FP32 = mybir.dt.float32
F32R = mybir.dt.float32r
BF16 = mybir.dt.bfloat16


def build_bass():
    nc = bacc.Bacc("TRN2", target_bir_lowering=False, debug=False,
                   num_devices=N_CORES)

    qa = nc.dram_tensor("qa", [ROUNDS, 128, S], BF16, kind="ExternalInput")
    qb = nc.dram_tensor("qb", [ROUNDS, DA, S], BF16, kind="ExternalInput")
    ka = nc.dram_tensor("ka", [ROUNDS, 128, KSLICE], BF16, kind="ExternalInput")
    kb = nc.dram_tensor("kb", [ROUNDS, DA, KSLICE], BF16, kind="ExternalInput")
    vm = nc.dram_tensor("vm", [ROUNDS, 128, KC, D + 3], FP32,
                        kind="ExternalInput")
    outp = nc.dram_tensor(
        "outp", [ROUNDS, D * S // N_CORES], FP32, kind="ExternalOutput"
    )

    ccin = nc.dram_tensor("ccin", [ROUNDS, D, S], FP32)
    ccout = nc.dram_tensor("ccout", [ROUNDS, D * S // N_CORES], FP32)

    with tile.TileContext(nc) as tc:
        with (
            tc.tile_pool(name="qp", bufs=2) as qp,
            tc.tile_pool(name="kp", bufs=2) as kp,
            tc.tile_pool(name="vp", bufs=2) as vp,
            tc.tile_pool(name="v2p", bufs=2) as v2p,
            tc.tile_pool(name="mp", bufs=2) as mp,
            tc.tile_pool(name="pp", bufs=2) as pp,
            tc.tile_pool(name="statp", bufs=2) as statp,
            tc.tile_pool(name="outp_sb", bufs=1) as outp_sb,
            tc.tile_pool(name="ps_s", bufs=2, space="PSUM") as ps_s,
            tc.tile_pool(name="ps_o", bufs=2, space="PSUM") as ps_o,
            tc.tile_pool(name="ps_b", bufs=1, space="PSUM") as ps_b,
        ):
            def emit_pass1_units(r):
                qa_t = qp.tile([128, S], BF16, tag="qa")
                nc.sync.dma_start(out=qa_t, in_=qa[r])
                qb_t = qp.tile([DA, S], BF16, tag="qb")
                nc.sync.dma_start(out=qb_t, in_=qb[r])
                ka_t = kp.tile([128, KSLICE], BF16, tag="ka")
                nc.sync.dma_start(out=ka_t, in_=ka[r])
                kb_t = kp.tile([DA, KSLICE], BF16, tag="kb")
                nc.sync.dma_start(out=kb_t, in_=kb[r])
                vm_t = vp.tile([128, KC, D + 3], FP32, tag="vm")
                nc.sync.dma_start(out=vm_t, in_=vm[r])
                v_t = vm_t[:, :, :D]
                mf_t = vm_t[:, :, D]
                ma_t = vm_t[:, :, D + 1]
                mc_t = vm_t[:, :, D + 2]

                p_t = pp.tile([128, KC, S], F32R, tag="P")
                lpart = statp.tile([128, KC, NQ], FP32, tag="lpart")
                state = {}
                for kc in range(KC):
                    ksl = slice(kc * 128, (kc + 1) * 128)
                    for j in range(NQ):
                        ps = ps_s.tile([128, QTILE], FP32, tag="s")
                        for h2 in range(QTILE // 512):
                            q0 = j * QTILE + h2 * 512
                            out_sl = ps[:, h2 * 512:(h2 + 1) * 512]
                            nc.tensor.matmul(
                                out_sl,
                                lhsT=ka_t[:, ksl],
                                rhs=qa_t[:, q0:q0 + 512],
                                start=True,
                                stop=False,
                            )
                            nc.tensor.matmul(
                                out_sl,
                                lhsT=kb_t[:, ksl],
                                rhs=qb_t[:, q0:q0 + 512],
                                start=False,
                                stop=True,
                            )
                        nc.scalar.activation(
                            p_t[:, kc, j * QTILE:(j + 1) * QTILE],
                            ps,
                            mybir.ActivationFunctionType.Exp,
                            accum_out=lpart[:, kc, j:j + 1],
                        )
                        yield None

                # normalizer: r = mask / l  (masked k -> 0)
                l_t = statp.tile([128, KC], FP32, tag="l")
                nc.vector.tensor_reduce(
                    l_t, lpart, axis=mybir.AxisListType.X, op=mybir.AluOpType.add
                )
                denom = statp.tile([128, KC], FP32, tag="denom")
                nc.vector.tensor_add(denom, l_t, ma_t)  # masked k: l=0 -> denom=1
                rec = statp.tile([128, KC], FP32, tag="rec")
                nc.vector.reciprocal(rec, denom)
                r_t = statp.tile([128, KC], FP32, tag="r")
                nc.vector.tensor_mul(r_t, rec, mf_t)

                v2_t = v2p.tile([128, KC, D], F32R, tag="v2")
                for kc in range(KC):
                    nc.vector.tensor_scalar_mul(
                        v2_t[:, kc, :], v_t[:, kc, :], r_t[:, kc:kc + 1]
                    )

                # bias: b[d] = sum_{local masked k} v[k,d] / S
                pb = ps_b.tile([D, 1], FP32, tag="b")
                for kc in range(KC):
                    nc.tensor.matmul(
                        pb,
                        lhsT=v_t[:, kc, :],
                        rhs=mc_t[:, kc:kc + 1],
                        start=(kc == 0),
                        stop=(kc == KC - 1),
                    )
                b_sb = statp.tile([D, 1], FP32, tag="bsb")
                nc.vector.tensor_copy(b_sb, pb)
                state["pvb"] = (p_t, v2_t, b_sb)
                yield state

            def emit_pass2_units(r, state):
                p_t, v2_t, b_sb = state["pvb"]
                out_t = outp_sb.tile([D, S], FP32, tag="out")
                for qq in range(S // 512):
                    po = ps_o.tile([D, 512], FP32, tag="o")
                    for kc in range(KC):
                        nc.tensor.matmul(
                            po,
                            lhsT=v2_t[:, kc, :],
                            rhs=p_t[:, kc, qq * 512:(qq + 1) * 512],
                            start=(kc == 0),
                            stop=(kc == KC - 1),
                        )
                    nc.vector.tensor_scalar_add(
                        out_t[:, qq * 512:(qq + 1) * 512], po, b_sb
                    )
                    yield None

                nc.sync.dma_start(out=ccin[r], in_=out_t)
                nc.gpsimd.collective_compute(
                    "ReduceScatter",
                    mybir.AluOpType.add,
                    replica_groups=[list(range(N_CORES))],
                    ins=[ccin[r]],
                    outs=[ccout[r]],
                )
                nc.sync.dma_start(out=outp[r], in_=ccout[r])
                yield None

            # software pipeline with fine-grained interleave: round r's score
            # matmuls (bf16, HAM-visible) are woven between round r-1's output
            # matmuls (fp32r, which do not register as PE activity) so the
            # clock gate stays warm.
            def drain(gen):
                for _ in gen:
                    pass

            prev_state = None
            for r in range(ROUNDS):
                g1 = emit_pass1_units(r)
                g2 = (
                    emit_pass2_units(r - 1, prev_state)
                    if prev_state is not None else None
                )
                state = None
                n1 = 0
                while True:
                    item = next(g1, "DONE")
                    if item == "DONE":
                        break
                    if isinstance(item, dict):
                        state = item
                    n1 += 1
                    # consume pass2 units only in the second half of pass1:
                    # the first output matmul waits on round r-1's full
                    # exp->stats->v'' chain, and the PE queue is in-order --
                    # injecting it too early head-of-line-blocks the score
                    # matmuls behind it.
                    if g2 is not None and n1 >= 8:
                        next(g2, None)
                if g2 is not None:
                    drain(g2)
                prev_state = state
            drain(emit_pass2_units(ROUNDS - 1, prev_state))

    nc.finalize()
    return nc


_NC_CACHE = None


def _get_nc():
    global _NC_CACHE
    if _NC_CACHE is None:
        _NC_CACHE = build_bass()
    return _NC_CACHE


def kernel(q, k, v, attn_mask):
    q = np.asarray(q, dtype=np.float32)
    k = np.asarray(k, dtype=np.float32)
    v = np.asarray(v, dtype=np.float32)
    mask = np.asarray(attn_mask).astype(bool)[0]          # [S]

    bf = ml_dtypes.bfloat16
    maskf = mask.astype(np.float32)                        # 1.0 keep / 0.0 drop
    cvec = np.where(mask, np.float32(0.0), np.float32(NEG)).astype(bf)
    ones = np.ones(S, bf)

    qt = np.ascontiguousarray(q[0].transpose(0, 2, 1))     # [H, D, S] fp32
    kt = np.ascontiguousarray(k[0].transpose(0, 2, 1))
    qh = qt.astype(bf)
    ql = (qt - qh.astype(np.float32)).astype(bf)
    kh = kt.astype(bf)
    kl = (kt - kh.astype(np.float32)).astype(bf)

    qa = np.concatenate([qh, ql], axis=1)                  # [H, 128, S]
    qb = np.empty((H, DA, S), bf)
    qb[:, :D] = qh
    qb[:, D] = cvec
    qb[:, D + 1] = ones
    ka_full = np.concatenate([kh, kh], axis=1)             # [H, 128, S]
    kb_full = np.empty((H, DA, S), bf)
    kb_full[:, :D] = kl
    kb_full[:, D] = ones
    kb_full[:, D + 1] = cvec

    mf_full = maskf.reshape(N_CORES, KC, 128).transpose(0, 2, 1)   # [8,128,KC]
    ma_full = 1.0 - mf_full
    mc_full = ma_full / np.float32(S)

    in_maps = []
    for c in range(N_CORES):
        sl = slice(c * KSLICE, (c + 1) * KSLICE)
        # packed [H, 128, KC, D+3]: v | maskf | 1-maskf | (1-maskf)/S
        vmc = np.empty((H, 128, KC, D + 3), np.float32)
        vmc[..., :D] = v[0, :, sl, :].reshape(H, KC, 128, D).transpose(0, 2, 1, 3)
        vmc[..., D] = mf_full[c][None]
        vmc[..., D + 1] = ma_full[c][None]
        vmc[..., D + 2] = mc_full[c][None]
        in_maps.append({
            "qa": qa,
            "qb": qb,
            "ka": np.ascontiguousarray(ka_full[:, :, sl]),
            "kb": np.ascontiguousarray(kb_full[:, :, sl]),
            "vm": vmc,
        })

    nc = _get_nc()
    trace = bool(os.environ.get("KERNEL_TRACE"))
    res = run_bass_kernel_spmd(
        nc, in_maps, list(range(N_CORES)),
        trace=trace,
        tmpdir=os.environ.get("KERNEL_TRACE_DIR") or None,
    )
    if trace and res.exec_time_ns is not None:
        print(f"HW exec time: {res.exec_time_ns} ns")

    # gather: core c holds rows [8c, 8c+8) of outT[h] = [D, S]
    out = np.empty((B, H, S, D), np.float32)
    rows = D // N_CORES
    for h in range(H):
        out_t = np.empty((D, S), np.float32)
        for c in range(N_CORES):
            out_t[c * rows:(c + 1) * rows] = (
                res.results[c]["outp"][h].reshape(rows, S)
            )
        out[0, h] = out_t.T
    return out
